# revision 1
# baseline (speedup 1.0000x reference)
"""Trainium2 Bass kernel for nn_AttentionWithTime (differential-attention block).

Sharding: data-parallel over batch B=8 -> one batch element per NeuronCore,
no collectives. Per-core single-batch block:

  qkv = LN(x) @ Wqkv + b; q1,q2,k1,k2 (64-dim heads), v (512-dim heads)
  attn = softmax(q1k1') - lam*softmax(q2k2'); out = attn @ v
  x2 = x + out @ Wm + bm
  tp = swish-MLP(t); h = LN(x2 + tp); y = x2 + swish(h@Wf1+bf1)@Wf2 + bf2

Layout strategy per core:
  - activations kept feature-major ("T" = [feature, token]) so natural
    [fan_in, fan_out] weight tiles serve directly as matmul lhsT
  - LN stats/normalize in token-major, then PE-transpose to feature-major
  - scores q-major [query(part), key(free)] -> softmax denom via ACT
    exp+accum_out; normalization by per-partition reciprocal scalars
  - combined attention weights PE-transposed for the attn@v matmul
  - merge and ffn2 projections emitted token-major (lhsT = feature-major
    activation tile) so residual adds need no transposes
  - q1/q2 (and k1/k2) packed in one 128-partition tile -> both diff-attn
    score matmuls run concurrently in the PE array (row groups 0-63/64-127)
  - v bias folded into attention output: rows of attn sum to (1-lam) exactly
  - bias vectors DMA'd as contiguous rows, turned into per-partition columns
    with tiny PE transposes (avoids 4-byte-strided DMA descriptors)
Matmul dtype bf16 (fp32 accumulate); residual/softmax paths fp32.
"""
import numpy as np

import concourse.bass as bass
import concourse.mybir as mybir
import concourse.tile as tile
from concourse import bacc
from concourse.masks import make_identity

B, N, D, H, DH, DE, DT = 8, 1024, 512, 8, 64, 2048, 256
DQKV = 2 * 512 + 2 * 512 + 4096  # 6144
NT = N // 128   # 8 token tiles
FT = D // 128   # 4 feature tiles
EPS = 1e-5
SCALE = DH ** -0.5

f32 = mybir.dt.float32
bf16 = mybir.dt.bfloat16
AF = mybir.ActivationFunctionType
ALU = mybir.AluOpType


def build_program(lam: float):
    nc = bacc.Bacc("TRN2", target_bir_lowering=False, debug=False, num_devices=8)

    x_d = nc.dram_tensor("x", [N, D], f32, kind="ExternalInput")
    t_d = nc.dram_tensor("t", [DT], f32, kind="ExternalInput")
    Wqkv_d = nc.dram_tensor("Wqkv", [D, DQKV], f32, kind="ExternalInput")
    bqkv_d = nc.dram_tensor("bqkv", [DQKV], f32, kind="ExternalInput")
    Wm_d = nc.dram_tensor("Wm", [4096, D], f32, kind="ExternalInput")
    bm_d = nc.dram_tensor("bm", [D], f32, kind="ExternalInput")
    Wt1_d = nc.dram_tensor("Wt1", [DT, DT], f32, kind="ExternalInput")
    bt1_d = nc.dram_tensor("bt1", [DT], f32, kind="ExternalInput")
    Wt2_d = nc.dram_tensor("Wt2", [DT, D], f32, kind="ExternalInput")
    bt2_d = nc.dram_tensor("bt2", [D], f32, kind="ExternalInput")
    Wf1_d = nc.dram_tensor("Wf1", [D, DE], f32, kind="ExternalInput")
    bf1_d = nc.dram_tensor("bf1", [DE], f32, kind="ExternalInput")
    Wf2_d = nc.dram_tensor("Wf2", [DE, D], f32, kind="ExternalInput")
    bf2_d = nc.dram_tensor("bf2", [D], f32, kind="ExternalInput")
    ln1g_d = nc.dram_tensor("ln1_g", [D], f32, kind="ExternalInput")
    ln1b_d = nc.dram_tensor("ln1_b", [D], f32, kind="ExternalInput")
    lnfg_d = nc.dram_tensor("lnf_g", [D], f32, kind="ExternalInput")
    lnfb_d = nc.dram_tensor("lnf_b", [D], f32, kind="ExternalInput")
    y_d = nc.dram_tensor("y", [N, D], f32, kind="ExternalOutput")

    with tile.TileContext(nc) as tc:
        _build(tc, lam, locals())
    nc.compile()
    return nc


def _build(tc, lam, d):
    nc = tc.nc
    x_d, t_d, y_d = d["x_d"], d["t_d"], d["y_d"]
    Wqkv_d, bqkv_d, Wm_d, bm_d = d["Wqkv_d"], d["bqkv_d"], d["Wm_d"], d["bm_d"]
    Wt1_d, bt1_d, Wt2_d, bt2_d = d["Wt1_d"], d["bt1_d"], d["Wt2_d"], d["bt2_d"]
    Wf1_d, bf1_d, Wf2_d, bf2_d = d["Wf1_d"], d["bf1_d"], d["Wf2_d"], d["bf2_d"]
    ln1g_d, ln1b_d, lnfg_d, lnfb_d = d["ln1g_d"], d["ln1b_d"], d["lnfg_d"], d["lnfb_d"]

    dma = nc.sync.dma_start

    from contextlib import ExitStack
    with ExitStack() as _es:
        cst = _es.enter_context(tc.tile_pool(name="cst", bufs=1))
        small = _es.enter_context(tc.tile_pool(name="small", bufs=8))
        maccp = _es.enter_context(tc.tile_pool(name="macc", bufs=NT))
        wf1p = _es.enter_context(tc.tile_pool(name="wf1bf", bufs=16))
        wf2p = _es.enter_context(tc.tile_pool(name="wf2bf", bufs=16))
        ps_s = _es.enter_context(tc.tile_pool(name="ps_s", bufs=2, space="PSUM"))
        ps_tr = _es.enter_context(tc.tile_pool(name="ps_tr", bufs=2, space="PSUM"))
        ps_sm = _es.enter_context(tc.tile_pool(name="ps_sm", bufs=2, space="PSUM"))
        # ---------- constants ----------
        ident = cst.tile([128, 128], f32)
        make_identity(nc, ident[:])
        ident_bf = cst.tile([128, 128], bf16)
        nc.gpsimd.tensor_copy(ident_bf[:], ident[:])
        ones1 = cst.tile([1, 128], f32)
        nc.gpsimd.memset(ones1[:], 1.0)
        eps_c = cst.tile([128, 1], f32)
        nc.gpsimd.memset(eps_c[:], EPS)

        def row_to_cols(row, dest, pairs):
            """pairs: list of (row_off, plen, dest_pbase, dest_col)."""
            for off, plen, pb, col in pairs:
                trp = ps_tr.tile([128, 128], f32, tag="tr", name="rc_tr")
                nc.tensor.transpose(trp[pb:pb + plen, 0:1], row[0:1, off:off + plen],
                                    ones1[0:1, 0:1])
                nc.vector.tensor_copy(dest[pb:pb + plen, col:col + 1],
                                      trp[pb:pb + plen, 0:1])

        # bias vectors as contiguous rows -> per-partition column tiles.
        # Rows live in a transient pool so their SBUF space is reclaimed.
        ln1g_c = cst.tile([128, FT], f32)
        ln1b_c = cst.tile([128, FT], f32)
        lnfg_c = cst.tile([128, FT], f32)
        lnfb_c = cst.tile([128, FT], f32)
        bf1_c = cst.tile([128, DE // 128], f32)
        bt1_c = cst.tile([128, DT // 128], f32)
        bqp = cst.tile([128, 16], f32)
        bvv = cst.tile([128, 32], f32)
        tT = cst.tile([128, 2], f32)
        bm_r = cst.tile([1, D], f32); dma(bm_r[:], bm_d[:].rearrange("(o k) -> o k", o=1))
        bt2_r = cst.tile([1, D], f32); dma(bt2_r[:], bt2_d[:].rearrange("(o k) -> o k", o=1))
        bf2_r = cst.tile([1, D], f32); dma(bf2_r[:], bf2_d[:].rearrange("(o k) -> o k", o=1))
        with tc.tile_pool(name="rows", bufs=1) as rows:
            bq_row = rows.tile([1, DQKV], f32)
            dma(bq_row[:], bqkv_d[:].rearrange("(o k) -> o k", o=1))
            lng_row = rows.tile([1, 4 * D], f32)
            for i, v_d in enumerate((ln1g_d, ln1b_d, lnfg_d, lnfb_d)):
                dma(lng_row[0:1, i * D:(i + 1) * D], v_d[:].rearrange("(o k) -> o k", o=1))
            bf1_row = rows.tile([1, DE], f32)
            dma(bf1_row[:], bf1_d[:].rearrange("(o k) -> o k", o=1))
            bt1_row = rows.tile([1, DT], f32)
            dma(bt1_row[:], bt1_d[:].rearrange("(o k) -> o k", o=1))
            t_row = rows.tile([1, DT], f32)
            dma(t_row[:], t_d[:].rearrange("(o k) -> o k", o=1))
            for i, dest in enumerate((ln1g_c, ln1b_c, lnfg_c, lnfb_c)):
                row_to_cols(lng_row, dest, [(i * D + j * 128, 128, 0, j) for j in range(FT)])
            row_to_cols(bf1_row, bf1_c, [(j * 128, 128, 0, j) for j in range(DE // 128)])
            row_to_cols(bt1_row, bt1_c, [(j * 128, 128, 0, j) for j in range(DT // 128)])
            # paired q/k bias columns: col p<8 -> [q1_h | q2_h], p>=8 -> [k1_h | k2_h]
            # (transpose outputs must start at psum partition 0, so assemble the
            # pair into a [1,128] staging row first)
            for p in range(16):
                h, base = p % 8, (0 if p < 8 else 1024)
                sr = rows.tile([1, 128], f32, tag="pair_row", bufs=3, name="pair_row")
                nc.gpsimd.tensor_copy(sr[0:1, 0:64], bq_row[0:1, base + h * 64:base + (h + 1) * 64])
                nc.gpsimd.tensor_copy(sr[0:1, 64:128], bq_row[0:1, base + 512 + h * 64:base + 512 + (h + 1) * 64])
                row_to_cols(sr, bqp, [(0, 128, 0, p)])
            # v bias columns scaled by (1-lam)
            row_to_cols(bq_row, bvv, [(2048 + j * 128, 128, 0, j) for j in range(32)])
            row_to_cols(t_row, tT, [(j * 128, 128, 0, j) for j in range(2)])
        nc.vector.tensor_scalar(bvv[:], bvv[:], 1.0 - lam, None, ALU.mult)

        # ---------- time MLP (tiny, fp32) ----------
        wt1 = [cst.tile([128, DT], f32, name=f"wt1_{i}", tag="wt1") for i in range(2)]
        for ft in range(2):
            dma(wt1[ft][:], Wt1_d[ft * 128:(ft + 1) * 128, :])
        wt2 = [cst.tile([128, D], f32, name=f"wt2_{i}", tag="wt2") for i in range(2)]
        for ft in range(2):
            dma(wt2[ft][:], Wt2_d[ft * 128:(ft + 1) * 128, :])
        s_cols = []
        for dc in range(2):
            l1_ps = ps_sm.tile([128, 512], f32, tag="sm")
            for ft in range(2):
                nc.tensor.matmul(l1_ps[:, 0:1], wt1[ft][:, dc * 128:(dc + 1) * 128],
                                 tT[:, ft:ft + 1], start=(ft == 0), stop=(ft == 1))
            sg_c = small.tile([128, 1], f32, tag="sg_col")
            nc.scalar.activation(sg_c[:], l1_ps[:, 0:1], AF.Sigmoid, bias=bt1_c[:, dc:dc + 1])
            s_c = small.tile([128, 1], f32, tag="s_col")
            nc.vector.scalar_tensor_tensor(s_c[:], l1_ps[:, 0:1], bt1_c[:, dc:dc + 1],
                                           sg_c[:], ALU.add, ALU.mult)
            s_cols.append(s_c)
        tp_ps = ps_sm.tile([1, 512], f32, tag="sm")
        for dc in range(2):
            nc.tensor.matmul(tp_ps[:], s_cols[dc][:], wt2[dc][:],
                             start=(dc == 0), stop=(dc == 1))
        # row1 = tp + bt2 + bm (pre-LNf add), row2 = bm + bf2 (final residual add)
        row1 = cst.tile([1, D], f32)
        nc.vector.tensor_add(row1[:], tp_ps[:], bt2_r[:])
        nc.vector.tensor_add(row1[:], row1[:], bm_r[:])
        row2 = cst.tile([1, D], f32)
        nc.vector.tensor_add(row2[:], bm_r[:], bf2_r[:])
        TP1 = cst.tile([128, D], f32)
        TP2 = cst.tile([128, D], f32)
        for row, TP in ((row1, TP1), (row2, TP2)):
            tp_b = ps_sm.tile([128, 512], f32, tag="sm")
            nc.tensor.matmul(tp_b[:], ones1[:], row[:], start=True, stop=True)
            nc.vector.tensor_copy(TP[:], tp_b[:])

        # ================= phase A: LN1, qk pairs, attention, merge =========
        with ExitStack() as _esA:
            xtmp = _esA.enter_context(tc.tile_pool(name="xtm", bufs=NT))
            lnxp = _esA.enter_context(tc.tile_pool(name="lnx", bufs=FT))
            qkp = _esA.enter_context(tc.tile_pool(name="qk", bufs=16))
            wst = _esA.enter_context(tc.tile_pool(name="wstage", bufs=5))
            wqkp = _esA.enter_context(tc.tile_pool(name="wqk", bufs=36))
            wbfp = _esA.enter_context(tc.tile_pool(name="wbf", bufs=8))
            normp = _esA.enter_context(tc.tile_pool(name="norm", bufs=3))
            vhp = _esA.enter_context(tc.tile_pool(name="vh", bufs=9))
            ewp = _esA.enter_context(tc.tile_pool(name="ew", bufs=4))
            utp = _esA.enter_context(tc.tile_pool(name="ut", bufs=16))
            otp = _esA.enter_context(tc.tile_pool(name="ot", bufs=4))
            # ---------- LN1 (token-major) + transpose to feature-major ------
            xtm = []
            lnxT = [lnxp.tile([128, N], bf16, name=f"lnxT_{i}", tag="lnxT") for i in range(FT)]
            for nt in range(NT):
                xt = xtmp.tile([128, D], f32, tag="xtm")
                dma(xt[:], x_d[nt * 128:(nt + 1) * 128, :])
                xtm.append(xt)
                st6 = small.tile([128, 6], f32, tag="st6")
                nc.vector.bn_stats(out=st6[:], in_=xt[:])
                mv = small.tile([128, 2], f32, tag="mv")
                nc.vector.bn_aggr(out=mv[:], in_=st6[:])
                rstd = small.tile([128, 1], f32, tag="rstd")
                nc.scalar.activation(rstd[:], mv[:, 1:2], AF.Sqrt, bias=eps_c[:])
                nc.vector.reciprocal(rstd[:], rstd[:])
                nm = small.tile([128, 1], f32, tag="nm")
                nc.vector.tensor_scalar(nm[:], mv[:, 0:1], rstd[:], -1.0, ALU.mult, ALU.mult)
                xn = normp.tile([128, D], f32, tag="norm")
                nc.scalar.activation(xn[:], xt[:], AF.Identity, bias=nm[:], scale=rstd[:])
                for ft in range(FT):
                    tr = ps_tr.tile([128, 128], f32, tag="tr")
                    nc.tensor.transpose(tr[:], xn[:, ft * 128:(ft + 1) * 128], ident[:])
                    nc.vector.tensor_scalar(
                        lnxT[ft][:, nt * 128:(nt + 1) * 128], tr[:],
                        ln1g_c[:, ft:ft + 1], ln1b_c[:, ft:ft + 1], ALU.mult, ALU.add)

            # ---------- qk pair projections (feature-major) ------------------
            # big contiguous W DMAs; pairing done by on-chip bf16 sub-copies.
            qk = []
            for half in range(2):            # 0: q pairs, 1: k pairs
                wpair = [[None] * FT for _ in range(8)]
                for ft in range(FT):
                    ca = wst.tile([128, 512], f32, tag="wf32", name="wqk_ca")
                    cb = wst.tile([128, 512], f32, tag="wf32", name="wqk_cb")
                    dma(ca[:], Wqkv_d[ft * 128:(ft + 1) * 128, half * 1024:half * 1024 + 512])
                    dma(cb[:], Wqkv_d[ft * 128:(ft + 1) * 128, half * 1024 + 512:half * 1024 + 1024])
                    for h in range(H):
                        wp = wqkp.tile([128, 128], bf16, tag="wqk_bf", name="wqk_bf")
                        nc.gpsimd.tensor_copy(wp[:, 0:64], ca[:, h * 64:(h + 1) * 64])
                        nc.gpsimd.tensor_copy(wp[:, 64:128], cb[:, h * 64:(h + 1) * 64])
                        wpair[h][ft] = wp
                for h in range(H):
                    p = half * 8 + h
                    qt = qkp.tile([128, N], bf16, tag="qk", name=f"qk_{p}")
                    for ch in range(2):
                        ps = ps_sm.tile([128, 512], f32, tag="sm")
                        for ft in range(FT):
                            nc.tensor.matmul(ps[:], wpair[h][ft][:],
                                             lnxT[ft][:, ch * 512:(ch + 1) * 512],
                                             start=(ft == 0), stop=(ft == FT - 1))
                        nc.vector.tensor_scalar(qt[:, ch * 512:(ch + 1) * 512], ps[:],
                                                bqp[:, p:p + 1], None, ALU.add)
                    qk.append(qt)

            # ---------- attention heads ------------------------------------
            wf1, wf2 = [], []
            for h in range(H):
                qp, kp = qk[h], qk[8 + h]
                # v_h token-major [m, 512]
                wv = []
                for ft in range(FT):
                    wvf = wst.tile([128, 512], f32, tag="wf32", name="wv_f32")
                    dma(wvf[:], Wqkv_d[ft * 128:(ft + 1) * 128, 2048 + h * 512:2048 + (h + 1) * 512])
                    wvb = wbfp.tile([128, 512], bf16, tag="wbf", name="wv_bf")
                    nc.gpsimd.tensor_copy(wvb[:], wvf[:])
                    wv.append(wvb)
                v_sb = []
                for mt in range(NT):
                    vps = ps_sm.tile([128, 512], f32, tag="sm")
                    for ft in range(FT):
                        nc.tensor.matmul(vps[:], lnxT[ft][:, mt * 128:(mt + 1) * 128], wv[ft][:],
                                         start=(ft == 0), stop=(ft == FT - 1))
                    vt = vhp.tile([128, 512], bf16, tag="vh")
                    nc.scalar.copy(vt[:], vps[:])
                    v_sb.append(vt)

                # scores / softmax / combine / transpose per query tile
                # UT chunked along keys' free dim for earlier O start
                UT = [[utp.tile([128, 512], bf16, tag="ut", name=f"UT_{ch}_{mt}")
                       for mt in range(NT)] for ch in range(2)]
                for nt in range(NT):
                    ch_n, sl_n = nt // 4, nt % 4
                    E1 = ewp.tile([128, N], bf16, tag="E")
                    E2 = ewp.tile([128, N], bf16, tag="E")
                    d1 = small.tile([128, 1], f32, tag="d1")
                    d2 = small.tile([128, 1], f32, tag="d2")
                    for mi, (E, dd, pb) in enumerate(((E1, d1, 0), (E2, d2, 64))):
                        S = ps_s.tile([128, N], f32, tag="S")
                        for mc in range(2):
                            nc.tensor.matmul(S[:, mc * 512:(mc + 1) * 512],
                                             qp[pb:pb + 64, nt * 128:(nt + 1) * 128],
                                             kp[pb:pb + 64, mc * 512:(mc + 1) * 512],
                                             start=True, stop=True)
                        nc.scalar.activation(E[:], S[:], AF.Exp, scale=SCALE,
                                             accum_out=dd[:])
                    rec1 = small.tile([128, 1], f32, tag="rec1")
                    nc.vector.reciprocal(rec1[:], d1[:])
                    r2l = small.tile([128, 1], f32, tag="r2l")
                    nc.vector.reciprocal(r2l[:], d2[:])
                    nc.vector.tensor_scalar(r2l[:], r2l[:], lam, None, ALU.mult)
                    E2p = ewp.tile([128, N], bf16, tag="Ew")
                    nc.gpsimd.tensor_scalar(E2p[:], E2[:], r2l[:], None, ALU.mult)
                    U = ewp.tile([128, N], bf16, tag="Ew")
                    nc.vector.scalar_tensor_tensor(U[:], E1[:], rec1[:], E2p[:],
                                                   ALU.mult, ALU.subtract)
                    for mt in range(NT):
                        trp = ps_tr.tile([128, 128], bf16, tag="tr")
                        nc.tensor.transpose(trp[:], U[:, mt * 128:(mt + 1) * 128], ident_bf[:])
                        nc.vector.tensor_copy(UT[ch_n][mt][:, sl_n * 128:(sl_n + 1) * 128], trp[:])

                # O^T = v^T @ U^T (feature-major out), v-bias folded via bvv
                OT = [otp.tile([128, N], bf16, tag="ot", name=f"OT_{i}") for i in range(FT)]
                for ch in range(2):
                    for ct in range(FT):
                        ops = ps_sm.tile([128, 512], f32, tag="sm")
                        for mt in range(NT):
                            nc.tensor.matmul(ops[:], v_sb[mt][:, ct * 128:(ct + 1) * 128],
                                             UT[ch][mt][:],
                                             start=(mt == 0), stop=(mt == NT - 1))
                        nc.scalar.activation(OT[ct][:, ch * 512:(ch + 1) * 512], ops[:],
                                             AF.Identity, bias=bvv[:, h * 4 + ct:h * 4 + ct + 1])

                # merge partial (token-major): macc[nt] (+)= [x +] O_h @ Wm_h
                wm = []
                for ft in range(FT):
                    wmf = wst.tile([128, 512], f32, tag="wf32", name="wm_f32")
                    dma(wmf[:], Wm_d[h * 512 + ft * 128:h * 512 + (ft + 1) * 128, :])
                    wmb = wbfp.tile([128, 512], bf16, tag="wbf", name="wm_bf")
                    nc.gpsimd.tensor_copy(wmb[:], wmf[:])
                    wm.append(wmb)
                if h == 1:
                    # ffn weight staging: emitted after head 0 so its DMA/cast
                    # work has lower priority than startup, overlaps heads 1-7
                    for cg in range(4):
                        for ft in range(FT):
                            wf = wst.tile([128, 512], f32, tag="wf32", name="wf1_f32")
                            nc.scalar.dma_start(wf[:], Wf1_d[ft * 128:(ft + 1) * 128, cg * 512:(cg + 1) * 512])
                            wfb = wf1p.tile([128, 512], bf16, tag="wf1bf", name="wf1_bf")
                            nc.gpsimd.tensor_copy(wfb[:], wf[:])
                            wf1.append(wfb)   # index cg*FT + ft
                    for ft2 in range(DE // 128):
                        wf = wst.tile([128, 512], f32, tag="wf32", name="wf2_f32")
                        nc.scalar.dma_start(wf[:], Wf2_d[ft2 * 128:(ft2 + 1) * 128, :])
                        wfb = wf2p.tile([128, 512], bf16, tag="wf2bf", name="wf2_bf")
                        nc.gpsimd.tensor_copy(wfb[:], wf[:])
                        wf2.append(wfb)
                for nt in range(NT):
                    mps = ps_sm.tile([128, 512], f32, tag="sm")
                    for ft in range(FT):
                        nc.tensor.matmul(mps[:], OT[ft][:, nt * 128:(nt + 1) * 128], wm[ft][:],
                                         start=(ft == 0), stop=(ft == FT - 1))
                    if h == 0:
                        mt_ = maccp.tile([128, D], f32, tag="macc", name=f"macc_{nt}")
                        nc.vector.tensor_add(mt_[:], xtm[nt][:], mps[:])
                        if nt == 0:
                            macc = []
                        macc.append(mt_)
                    else:
                        nc.vector.tensor_add(macc[nt][:], macc[nt][:], mps[:])

        # ================= phase B: LNf + FFN ===============================
        with ExitStack() as _esB:
            hTp = _esB.enter_context(tc.tile_pool(name="hT", bufs=FT))
            aTp = _esB.enter_context(tc.tile_pool(name="aT", bufs=18))
            normp2 = _esB.enter_context(tc.tile_pool(name="norm2", bufs=3))
            yp = _esB.enter_context(tc.tile_pool(name="yp", bufs=3))
            hT = [hTp.tile([128, N], bf16, name=f"hT_{i}", tag="hT") for i in range(FT)]
            for nt in range(NT):
                x2p = normp2.tile([128, D], f32, tag="x2p")
                nc.vector.tensor_add(x2p[:], macc[nt][:], TP1[:])
                # macc becomes the final-residual base x2 + bm + bf2 (in place)
                nc.vector.tensor_add(macc[nt][:], macc[nt][:], TP2[:])
                st6 = small.tile([128, 6], f32, tag="st6")
                nc.vector.bn_stats(out=st6[:], in_=x2p[:])
                mv = small.tile([128, 2], f32, tag="mv")
                nc.vector.bn_aggr(out=mv[:], in_=st6[:])
                rstd = small.tile([128, 1], f32, tag="rstd")
                nc.scalar.activation(rstd[:], mv[:, 1:2], AF.Sqrt, bias=eps_c[:])
                nc.vector.reciprocal(rstd[:], rstd[:])
                nm = small.tile([128, 1], f32, tag="nm")
                nc.vector.tensor_scalar(nm[:], mv[:, 0:1], rstd[:], -1.0, ALU.mult, ALU.mult)
                hn = normp2.tile([128, D], f32, tag="hn")
                nc.scalar.activation(hn[:], x2p[:], AF.Identity, bias=nm[:], scale=rstd[:])
                for ft in range(FT):
                    tr = ps_tr.tile([128, 128], f32, tag="tr")
                    nc.tensor.transpose(tr[:], hn[:, ft * 128:(ft + 1) * 128], ident[:])
                    nc.vector.tensor_scalar(
                        hT[ft][:, nt * 128:(nt + 1) * 128], tr[:],
                        lnfg_c[:, ft:ft + 1], lnfb_c[:, ft:ft + 1], ALU.mult, ALU.add)

            # ffn1: aT[dc] = silu(hT' W1 + b1), feature-major [2048, N]
            aT = []
            for cg in range(4):
                for dc in range(4):
                    at = aTp.tile([128, N], bf16, tag="aT", name=f"aT_{cg}_{dc}")
                    for ch in range(2):
                        ps = ps_sm.tile([128, 512], f32, tag="sm")
                        for ft in range(FT):
                            nc.tensor.matmul(ps[:], wf1[cg * FT + ft][:, dc * 128:(dc + 1) * 128],
                                             hT[ft][:, ch * 512:(ch + 1) * 512],
                                             start=(ft == 0), stop=(ft == FT - 1))
                        gc = cg * 4 + dc
                        sg = aTp.tile([128, 512], bf16, tag="sg", bufs=3)
                        nc.scalar.activation(sg[:], ps[:], AF.Sigmoid, bias=bf1_c[:, gc:gc + 1])
                        nc.vector.scalar_tensor_tensor(at[:, ch * 512:(ch + 1) * 512], ps[:],
                                                       bf1_c[:, gc:gc + 1], sg[:],
                                                       ALU.add, ALU.mult)
                    aT.append(at)

            # ffn2 token-major + final residual
            for nt in range(NT):
                yps = ps_sm.tile([128, 512], f32, tag="sm")
                for ft2 in range(DE // 128):
                    nc.tensor.matmul(yps[:], aT[ft2][:, nt * 128:(nt + 1) * 128], wf2[ft2][:],
                                     start=(ft2 == 0), stop=(ft2 == DE // 128 - 1))
                yt = yp.tile([128, D], f32, tag="y")
                nc.vector.tensor_add(yt[:], macc[nt][:], yps[:])
                dma(y_d[nt * 128:(nt + 1) * 128, :], yt[:])


_NC_CACHE = {}


def _get_nc(lam: float):
    key = float(lam)
    if key not in _NC_CACHE:
        _NC_CACHE[key] = build_program(key)
    return _NC_CACHE[key]


def kernel(**inputs) -> np.ndarray:
    from concourse.bass_utils import run_bass_kernel_spmd

    lam = float(np.asarray(inputs["lam"]))
    nc = _get_nc(lam)
    names = ["Wqkv", "bqkv", "Wm", "bm", "Wt1", "bt1", "Wt2", "bt2",
             "Wf1", "bf1", "Wf2", "bf2", "ln1_g", "ln1_b", "lnf_g", "lnf_b"]
    shared = {n: np.ascontiguousarray(np.asarray(inputs[n], dtype=np.float32)) for n in names}
    x = np.asarray(inputs["x"], dtype=np.float32)
    t = np.asarray(inputs["t"], dtype=np.float32)
    in_maps = []
    for b in range(B):
        m = dict(shared)
        m["x"] = np.ascontiguousarray(x[b])
        m["t"] = np.ascontiguousarray(t[b])
        in_maps.append(m)
    res = run_bass_kernel_spmd(nc, in_maps, core_ids=list(range(B)))
    return np.stack([res.results[b]["y"] for b in range(B)], axis=0).astype(np.float32)

